# revision 3
# baseline (speedup 1.0000x reference)
"""Trainium2 Bass kernel for nn_DistillingLayer: per-channel shared-weight
Conv1d(k=3, stride=2, pad=1) + ELU + MaxPool1d(k=3, stride=2, pad=1) over
x:(16, 4096, 512) f32.

Strategy
--------
- Data-parallel over batch: 8 cores x 2 batches each. No communication.
- Layout: L lives in the SBUF *free* dimension. Each partition owns S=16
  consecutive L-rows (times D=512 channels) plus a 3-row halo, so the whole
  conv+pool dataflow is per-partition local (DVE lanes cannot cross
  partitions).
- ELU is monotonic, so maxpool commutes with it: pool the *pre-activation*
  conv outputs, then apply ELU once on the pooled result (half the L-rows).
  The conv bias folds into the conv pass itself.
- Conv outputs are computed de-interleaved (ce[m]=c[2m], co[m]=c[2m+1]) so
  every elementwise op runs on contiguous or simply-strided views:
      c[i] = w0*x[2i-1] + w1*x[2i] + w2*x[2i+1] + b
      out[m] = elu(max(co[m-1], ce[m], co[m]))
- ELU(v) = max(v, exp(min(v, 0)) - 1); min via relu(-v) on ScalarE with
  scale=-1, exp on ScalarE, final fused (e-1) max v on VectorE
  scalar_tensor_tensor.
- Weights/bias are baked as immediates (kernel() receives concrete values).
"""

import os
import sys

import numpy as np

for _p in ("/opt/trn_rl_repo", "/root/.axon_site/_ro/trn_rl_repo"):
    if os.path.isdir(_p) and _p not in sys.path:
        sys.path.append(_p)

import json as _json

import concourse.bass as bass
import concourse.bass2jax as bass2jax
import concourse.bass_utils as bass_utils
import concourse.mybir as mybir
from concourse.bass_utils import run_bass_kernel_spmd
from concourse.tile import TileContext

# ---------------------------------------------------------------------------
# Workaround: this container's walrus build rejects instructions carrying more
# than one sync wait ("Too many sync wait commands" in setupSyncWait). Tile's
# scheduler freely attaches several waits to one instruction, so post-process
# the BIR JSON before compile: hoist all but the last wait onto same-engine
# NoOps inserted just before the instruction (per-engine program order makes
# sequential waits equivalent to a multi-wait).
# ---------------------------------------------------------------------------

_orig_compile_bir_kernel = bass_utils.compile_bir_kernel


def _split_multi_waits(bir_json: bytes) -> bytes:
    j = _json.loads(bir_json)
    ctr = 0
    changed = False
    for fn in j["functions"]:
        for bb in fn["blocks"]:
            out = []
            for ins in bb["instructions"]:
                si = ins.get("sync_info")
                waits = (si.get("on_wait") or []) if si else []
                if len(waits) > 1:
                    changed = True
                    for w in waits[:-1]:
                        ctr += 1
                        out.append(
                            {
                                "debug": ins.get("debug", 0),
                                "engine": ins["engine"],
                                "ins": [],
                                "outs": [],
                                "name": f"waitsplit-{ctr}",
                                "opcode": "NoOp",
                                "text_hint": "waitsplit",
                                "sync_info": {"on_update": [], "on_wait": [w]},
                            }
                        )
                    si["on_wait"] = [waits[-1]]
                out.append(ins)
            bb["instructions"] = out
    if not changed:
        return bir_json
    return _json.dumps(j).encode()


def _patched_compile_bir_kernel(bir_json, tmpdir, neff_name="file.neff"):
    return _orig_compile_bir_kernel(_split_multi_waits(bir_json), tmpdir, neff_name)


bass_utils.compile_bir_kernel = _patched_compile_bir_kernel
bass2jax.compile_bir_kernel = _patched_compile_bir_kernel

# ---------------------------------------------------------------------------

N_CORES = 8
B, L, D = 16, 4096, 512
BPC = B // N_CORES  # batches per core
S = 16              # L-rows per partition per tile
T = L // (128 * S)  # tiles per batch
LC = L // 2         # conv output length
LP = LC // 2        # pool output length
JT = S // 4         # pool-output rows per partition per tile

F32 = mybir.dt.float32
ALU = mybir.AluOpType
AF = mybir.ActivationFunctionType

_cache: dict = {}

# Exposed for test harnesses: the BassKernelResults of the last run.
LAST_RESULT = None


def _build(w0: float, w1: float, w2: float, bias: float) -> bass.Bass:
    nc = bass.Bass()
    x = nc.dram_tensor("x", [BPC, L, D], F32, kind="ExternalInput")
    y = nc.dram_tensor("y", [BPC, LP, D], F32, kind="ExternalOutput")

    # Register the conv bias as a const AP so activation(bias=<float>) works
    # (only 0.0/1.0 are pre-registered). Same pattern as Bass.__init__.
    if (F32, bias) not in nc.const_aps.aps:
        bias_t = nc.alloc_sbuf_tensor(f"const-bias", [128, 1], F32)
        nc.gpsimd.memset(bias_t.ap(), bias)
        nc.const_aps.aps[(F32, bias)] = bias_t.ap()
        nc.all_engine_barrier()

    xrow = D              # elements per L-row
    xbat = L * D          # elements per input batch
    ybat = LP * D

    with TileContext(nc) as tc:
        with (
            tc.tile_pool(name="xp", bufs=3) as xp,
            tc.tile_pool(name="cep", bufs=2) as cep,
            tc.tile_pool(name="cop", bufs=2) as cop,
            tc.tile_pool(name="rp", bufs=2) as rp,
            tc.tile_pool(name="op", bufs=2) as op,
        ):
            for b in range(BPC):
                for t in range(T):
                    X = xp.tile([128, (S + 3) * D], F32)
                    base = b * xbat + (t * 128 * S - 3) * xrow
                    if t == 0:
                        # partition 0 would read rows -3..-1; load it
                        # separately and zero its halo (only x[-1] matters
                        # for the conv; rows -3/-2 feed co[-1], which is
                        # overwritten with -inf below).
                        nc.sync.dma_start(
                            out=X[1:128, :],
                            in_=bass.AP(
                                x,
                                base + S * xrow,
                                [[S * xrow, 127], [1, (S + 3) * xrow]],
                            ),
                        )
                        nc.sync.dma_start(
                            out=X[0:1, 3 * D :],
                            in_=bass.AP(x, b * xbat, [[S * xrow, 1], [1, S * xrow]]),
                        )
                        nc.gpsimd.memset(X[0:1, 0 : 3 * D], 0.0)
                    else:
                        nc.sync.dma_start(
                            out=X[:, :],
                            in_=bass.AP(
                                x, base, [[S * xrow, 128], [1, (S + 3) * xrow]]
                            ),
                        )

                    Xv = X[:, :].rearrange("p (r d) -> p r d", d=D)
                    # ce taps: x rows (local) 4j+2, 4j+3, 4j+4   j in [0, JT)
                    xa_e = Xv[:, 2 : 2 + 4 * JT - 3 : 4, :]
                    xb_e = Xv[:, 3 : 3 + 4 * JT - 3 : 4, :]
                    xc_e = Xv[:, 4 : 4 + 4 * JT - 3 : 4, :]
                    # co taps: x rows (local) 4k, 4k+1, 4k+2     k in [0, JT]
                    xa_o = Xv[:, 0 : 4 * JT + 1 : 4, :]
                    xb_o = Xv[:, 1 : 4 * JT + 2 : 4, :]
                    xc_o = Xv[:, 2 : 4 * JT + 3 : 4, :]

                    CE = cep.tile([128, JT * D], F32)
                    CO = cop.tile([128, (JT + 1) * D], F32)
                    ce3 = CE[:, :].rearrange("p (j d) -> p j d", d=D)
                    co3 = CO[:, :].rearrange("p (j d) -> p j d", d=D)

                    # conv (bias folded in): c = w0*xa + w1*xb + w2*xc + bias
                    nc.scalar.activation(ce3, xa_e, AF.Identity, bias=bias, scale=w0)
                    nc.vector.scalar_tensor_tensor(
                        ce3, xb_e, w1, ce3, op0=ALU.mult, op1=ALU.add
                    )
                    nc.vector.scalar_tensor_tensor(
                        ce3, xc_e, w2, ce3, op0=ALU.mult, op1=ALU.add
                    )
                    nc.scalar.activation(co3, xa_o, AF.Identity, bias=bias, scale=w0)
                    nc.vector.scalar_tensor_tensor(
                        co3, xb_o, w1, co3, op0=ALU.mult, op1=ALU.add
                    )
                    nc.vector.scalar_tensor_tensor(
                        co3, xc_o, w2, co3, op0=ALU.mult, op1=ALU.add
                    )
                    if t == 0:
                        # left pool pad: co[-1] = -inf
                        nc.gpsimd.memset(CO[0:1, 0:D], float("-inf"))

                    # maxpool (pre-activation; ELU is monotonic)
                    nc.vector.tensor_tensor(
                        CE[:, :], CE[:, :], CO[:, D:], op=ALU.max
                    )
                    nc.vector.tensor_tensor(
                        CE[:, :], CE[:, :], CO[:, : JT * D], op=ALU.max
                    )

                    # ELU(v) = max(v, exp(min(v,0)) - 1)
                    R = rp.tile([128, JT * D], F32)
                    nc.scalar.activation(R[:, :], CE[:, :], AF.Relu, scale=-1.0)
                    nc.scalar.activation(R[:, :], R[:, :], AF.Exp, scale=-1.0)
                    OUT = op.tile([128, JT * D], F32)
                    nc.vector.scalar_tensor_tensor(
                        OUT[:, :], R[:, :], -1.0, CE[:, :], op0=ALU.add, op1=ALU.max
                    )

                    nc.gpsimd.dma_start(
                        out=bass.AP(
                            y,
                            b * ybat + t * 128 * JT * D,
                            [[JT * D, 128], [1, JT * D]],
                        ),
                        in_=OUT[:, :],
                    )
    return nc


def kernel(x: np.ndarray, w: np.ndarray, b: np.ndarray) -> np.ndarray:
    global LAST_RESULT
    w = np.asarray(w, dtype=np.float32)
    bb = np.asarray(b, dtype=np.float32)
    key = (float(w[0]), float(w[1]), float(w[2]), float(bb[0]))
    if key not in _cache:
        _cache[key] = _build(*key)
    nc = _cache[key]

    x = np.ascontiguousarray(np.asarray(x, dtype=np.float32))
    in_maps = [
        {"x": np.ascontiguousarray(x[c * BPC : (c + 1) * BPC])}
        for c in range(N_CORES)
    ]
    res = run_bass_kernel_spmd(nc, in_maps, core_ids=list(range(N_CORES)))
    LAST_RESULT = res
    return np.concatenate([r["y"] for r in res.results], axis=0)


# revision 4
# speedup vs baseline: 1.0872x; 1.0872x over previous
"""Trainium2 Bass kernel for nn_DistillingLayer: per-channel shared-weight
Conv1d(k=3, stride=2, pad=1) + ELU + MaxPool1d(k=3, stride=2, pad=1) over
x:(16, 4096, 512) f32.

Strategy
--------
- Data-parallel over batch: 8 cores x 2 batches each. No communication.
- Layout: L lives in the SBUF *free* dimension. Each partition owns S=16
  consecutive L-rows (times D=512 channels) plus a 3-row halo, so the whole
  conv+pool dataflow is per-partition local (DVE lanes cannot cross
  partitions).
- ELU is monotonic, so maxpool commutes with it: pool the *pre-activation*
  conv outputs, then apply ELU once on the pooled result (half the L-rows).
  The conv bias folds into the conv pass itself.
- Conv outputs are computed de-interleaved (ce[m]=c[2m], co[m]=c[2m+1]) so
  every elementwise op runs on contiguous or simply-strided views:
      c[i] = w0*x[2i-1] + w1*x[2i] + w2*x[2i+1] + b
      out[m] = elu(max(co[m-1], ce[m], co[m]))
- ELU(v) = max(v, exp(min(v, 0)) - 1); min via relu(-v) on ScalarE with
  scale=-1, exp on ScalarE, final fused (e-1) max v on VectorE
  scalar_tensor_tensor.
- Weights/bias are baked as immediates (kernel() receives concrete values).
"""

import os
import sys

import numpy as np

for _p in ("/opt/trn_rl_repo", "/root/.axon_site/_ro/trn_rl_repo"):
    if os.path.isdir(_p) and _p not in sys.path:
        sys.path.append(_p)

import json as _json

import concourse.bass as bass
import concourse.bass2jax as bass2jax
import concourse.bass_utils as bass_utils
import concourse.mybir as mybir
from concourse.bass_utils import run_bass_kernel_spmd
from concourse.tile import TileContext

# ---------------------------------------------------------------------------
# Workaround: this container's walrus build rejects instructions carrying more
# than one sync wait ("Too many sync wait commands" in setupSyncWait). Tile's
# scheduler freely attaches several waits to one instruction, so post-process
# the BIR JSON before compile: hoist all but the last wait onto same-engine
# NoOps inserted just before the instruction (per-engine program order makes
# sequential waits equivalent to a multi-wait).
# ---------------------------------------------------------------------------

_orig_compile_bir_kernel = bass_utils.compile_bir_kernel


def _split_multi_waits(bir_json: bytes) -> bytes:
    j = _json.loads(bir_json)
    ctr = 0
    changed = False
    for fn in j["functions"]:
        for bb in fn["blocks"]:
            out = []
            for ins in bb["instructions"]:
                si = ins.get("sync_info")
                waits = (si.get("on_wait") or []) if si else []
                if len(waits) > 1:
                    changed = True
                    for w in waits[:-1]:
                        ctr += 1
                        out.append(
                            {
                                "debug": ins.get("debug", 0),
                                "engine": ins["engine"],
                                "ins": [],
                                "outs": [],
                                "name": f"waitsplit-{ctr}",
                                "opcode": "NoOp",
                                "text_hint": "waitsplit",
                                "sync_info": {"on_update": [], "on_wait": [w]},
                            }
                        )
                    si["on_wait"] = [waits[-1]]
                out.append(ins)
            bb["instructions"] = out
    if not changed:
        return bir_json
    return _json.dumps(j).encode()


def _patched_compile_bir_kernel(bir_json, tmpdir, neff_name="file.neff"):
    return _orig_compile_bir_kernel(_split_multi_waits(bir_json), tmpdir, neff_name)


bass_utils.compile_bir_kernel = _patched_compile_bir_kernel
bass2jax.compile_bir_kernel = _patched_compile_bir_kernel

# ---------------------------------------------------------------------------

N_CORES = 8
B, L, D = 16, 4096, 512
BPC = B // N_CORES  # batches per core
S = 16              # L-rows per partition per tile
T = L // (128 * S)  # tiles per batch
LC = L // 2         # conv output length
LP = LC // 2        # pool output length
JT = S // 4         # pool-output rows per partition per tile

F32 = mybir.dt.float32
ALU = mybir.AluOpType
AF = mybir.ActivationFunctionType

_cache: dict = {}

# Exposed for test harnesses: the BassKernelResults of the last run.
LAST_RESULT = None


def _build(w0: float, w1: float, w2: float, bias: float) -> bass.Bass:
    nc = bass.Bass()
    x = nc.dram_tensor("x", [BPC, L, D], F32, kind="ExternalInput")
    y = nc.dram_tensor("y", [BPC, LP, D], F32, kind="ExternalOutput")

    # Register the conv bias as a const AP so activation(bias=<float>) works
    # (only 0.0/1.0 are pre-registered). Same pattern as Bass.__init__.
    if (F32, bias) not in nc.const_aps.aps:
        bias_t = nc.alloc_sbuf_tensor(f"const-bias", [128, 1], F32)
        nc.gpsimd.memset(bias_t.ap(), bias)
        nc.const_aps.aps[(F32, bias)] = bias_t.ap()
        nc.all_engine_barrier()

    xrow = D              # elements per L-row
    xbat = L * D          # elements per input batch
    ybat = LP * D

    with TileContext(nc) as tc:
        with (
            tc.tile_pool(name="xp", bufs=3) as xp,
            tc.tile_pool(name="cep", bufs=2) as cep,
            tc.tile_pool(name="cop", bufs=2) as cop,
            tc.tile_pool(name="rp", bufs=2) as rp,
            tc.tile_pool(name="op", bufs=2) as op,
        ):
            for b in range(BPC):
                for t in range(T):
                    X = xp.tile([128, (S + 3) * D], F32)
                    base = b * xbat + (t * 128 * S - 3) * xrow
                    if t == 0:
                        # partition 0 would read rows -3..-1; load it
                        # separately and zero its halo (only x[-1] matters
                        # for the conv; rows -3/-2 feed co[-1], which is
                        # overwritten with -inf below).
                        nc.gpsimd.dma_start(
                            out=X[1:128, :],
                            in_=bass.AP(
                                x,
                                base + S * xrow,
                                [[S * xrow, 127], [1, (S + 3) * xrow]],
                            ),
                        )
                        nc.gpsimd.dma_start(
                            out=X[0:1, 3 * D :],
                            in_=bass.AP(x, b * xbat, [[S * xrow, 1], [1, S * xrow]]),
                        )
                        nc.gpsimd.memset(X[0:1, 0 : 3 * D], 0.0)
                    else:
                        nc.gpsimd.dma_start(
                            out=X[:, :],
                            in_=bass.AP(
                                x, base, [[S * xrow, 128], [1, (S + 3) * xrow]]
                            ),
                        )

                    Xv = X[:, :].rearrange("p (r d) -> p r d", d=D)
                    # ce taps: x rows (local) 4j+2, 4j+3, 4j+4   j in [0, JT)
                    xa_e = Xv[:, 2 : 2 + 4 * JT - 3 : 4, :]
                    xb_e = Xv[:, 3 : 3 + 4 * JT - 3 : 4, :]
                    xc_e = Xv[:, 4 : 4 + 4 * JT - 3 : 4, :]
                    # co taps: x rows (local) 4k, 4k+1, 4k+2     k in [0, JT]
                    xa_o = Xv[:, 0 : 4 * JT + 1 : 4, :]
                    xb_o = Xv[:, 1 : 4 * JT + 2 : 4, :]
                    xc_o = Xv[:, 2 : 4 * JT + 3 : 4, :]

                    CE = cep.tile([128, JT * D], F32)
                    CO = cop.tile([128, (JT + 1) * D], F32)
                    ce3 = CE[:, :].rearrange("p (j d) -> p j d", d=D)
                    co3 = CO[:, :].rearrange("p (j d) -> p j d", d=D)

                    # conv (bias folded in): c = w0*xa + w1*xb + w2*xc + bias
                    nc.scalar.activation(ce3, xa_e, AF.Identity, bias=bias, scale=w0)
                    nc.vector.scalar_tensor_tensor(
                        ce3, xb_e, w1, ce3, op0=ALU.mult, op1=ALU.add
                    )
                    nc.vector.scalar_tensor_tensor(
                        ce3, xc_e, w2, ce3, op0=ALU.mult, op1=ALU.add
                    )
                    nc.scalar.activation(co3, xa_o, AF.Identity, bias=bias, scale=w0)
                    nc.vector.scalar_tensor_tensor(
                        co3, xb_o, w1, co3, op0=ALU.mult, op1=ALU.add
                    )
                    nc.vector.scalar_tensor_tensor(
                        co3, xc_o, w2, co3, op0=ALU.mult, op1=ALU.add
                    )
                    if t == 0:
                        # left pool pad: co[-1] = -inf
                        nc.gpsimd.memset(CO[0:1, 0:D], float("-inf"))

                    # maxpool (pre-activation; ELU is monotonic)
                    nc.vector.tensor_tensor(
                        CE[:, :], CE[:, :], CO[:, D:], op=ALU.max
                    )
                    nc.vector.tensor_tensor(
                        CE[:, :], CE[:, :], CO[:, : JT * D], op=ALU.max
                    )

                    # ELU(v) = max(v, exp(min(v,0)) - 1)
                    R = rp.tile([128, JT * D], F32)
                    nc.scalar.activation(R[:, :], CE[:, :], AF.Relu, scale=-1.0)
                    nc.scalar.activation(R[:, :], R[:, :], AF.Exp, scale=-1.0)
                    OUT = op.tile([128, JT * D], F32)
                    nc.vector.scalar_tensor_tensor(
                        OUT[:, :], R[:, :], -1.0, CE[:, :], op0=ALU.add, op1=ALU.max
                    )

                    nc.gpsimd.dma_start(
                        out=bass.AP(
                            y,
                            b * ybat + t * 128 * JT * D,
                            [[JT * D, 128], [1, JT * D]],
                        ),
                        in_=OUT[:, :],
                    )
    return nc


def kernel(x: np.ndarray, w: np.ndarray, b: np.ndarray) -> np.ndarray:
    global LAST_RESULT
    w = np.asarray(w, dtype=np.float32)
    bb = np.asarray(b, dtype=np.float32)
    key = (float(w[0]), float(w[1]), float(w[2]), float(bb[0]))
    if key not in _cache:
        _cache[key] = _build(*key)
    nc = _cache[key]

    x = np.ascontiguousarray(np.asarray(x, dtype=np.float32))
    in_maps = [
        {"x": np.ascontiguousarray(x[c * BPC : (c + 1) * BPC])}
        for c in range(N_CORES)
    ]
    res = run_bass_kernel_spmd(nc, in_maps, core_ids=list(range(N_CORES)))
    LAST_RESULT = res
    return np.concatenate([r["y"] for r in res.results], axis=0)


# revision 8
# speedup vs baseline: 4.8386x; 4.4505x over previous
"""Trainium2 Bass kernel for nn_DistillingLayer: per-channel shared-weight
Conv1d(k=3, stride=2, pad=1) + ELU + MaxPool1d(k=3, stride=2, pad=1) over
x:(16, 4096, 512) f32.

Strategy
--------
- Data-parallel over batch: 8 cores x 2 batches each. No communication.
- Layout: L lives in the SBUF *free* dimension. Each partition owns S=16
  consecutive L-rows (times D=512 channels) plus a 3-row halo, so the whole
  conv+pool dataflow is per-partition local (DVE lanes cannot cross
  partitions).
- ELU is monotonic, so maxpool commutes with it: pool the *pre-activation*
  conv outputs, then apply ELU once on the pooled result (half the L-rows).
  The conv bias folds into the conv pass itself.
- Conv outputs are computed de-interleaved (ce[m]=c[2m], co[m]=c[2m+1]) so
  every elementwise op runs on contiguous or simply-strided views:
      c[i] = w0*x[2i-1] + w1*x[2i] + w2*x[2i+1] + b
      out[m] = elu(max(co[m-1], ce[m], co[m]))
- ELU(v) = max(v, exp(min(v, 0)) - 1); min via relu(-v) on ScalarE with
  scale=-1, exp on ScalarE, final fused (e-1) max v on VectorE
  scalar_tensor_tensor.
- Weights/bias are baked as immediates (kernel() receives concrete values).
"""

import os
import sys

import numpy as np

for _p in ("/opt/trn_rl_repo", "/root/.axon_site/_ro/trn_rl_repo"):
    if os.path.isdir(_p) and _p not in sys.path:
        sys.path.append(_p)

import json as _json

import concourse.bass as bass
import concourse.bass2jax as bass2jax
import concourse.bass_utils as bass_utils
import concourse.mybir as mybir
from concourse.bass_utils import run_bass_kernel_spmd
from concourse.tile import TileContext

# ---------------------------------------------------------------------------
# Workaround: this container's walrus build rejects instructions carrying more
# than one sync wait ("Too many sync wait commands" in setupSyncWait). Tile's
# scheduler freely attaches several waits to one instruction, so post-process
# the BIR JSON before compile: hoist all but the last wait onto same-engine
# NoOps inserted just before the instruction (per-engine program order makes
# sequential waits equivalent to a multi-wait).
# ---------------------------------------------------------------------------

_orig_compile_bir_kernel = bass_utils.compile_bir_kernel


def _split_multi_waits(bir_json: bytes) -> bytes:
    j = _json.loads(bir_json)
    ctr = 0
    changed = False
    for fn in j["functions"]:
        for bb in fn["blocks"]:
            out = []
            for ins in bb["instructions"]:
                si = ins.get("sync_info")
                waits = (si.get("on_wait") or []) if si else []
                if len(waits) > 1:
                    changed = True
                    for w in waits[:-1]:
                        ctr += 1
                        out.append(
                            {
                                "debug": ins.get("debug", 0),
                                "engine": ins["engine"],
                                "ins": [],
                                "outs": [],
                                "name": f"waitsplit-{ctr}",
                                "opcode": "NoOp",
                                "text_hint": "waitsplit",
                                "sync_info": {"on_update": [], "on_wait": [w]},
                            }
                        )
                    si["on_wait"] = [waits[-1]]
                out.append(ins)
            bb["instructions"] = out
    if not changed:
        return bir_json
    return _json.dumps(j).encode()


def _patched_compile_bir_kernel(bir_json, tmpdir, neff_name="file.neff"):
    return _orig_compile_bir_kernel(_split_multi_waits(bir_json), tmpdir, neff_name)


bass_utils.compile_bir_kernel = _patched_compile_bir_kernel
bass2jax.compile_bir_kernel = _patched_compile_bir_kernel

# ---------------------------------------------------------------------------

N_CORES = 8
B, L, D = 16, 4096, 512
BPC = B // N_CORES  # batches per core
S = 16              # L-rows per partition per tile
T = L // (128 * S)  # tiles per batch
LC = L // 2         # conv output length
LP = LC // 2        # pool output length
JT = S // 4         # pool-output rows per partition per tile

F32 = mybir.dt.float32
ALU = mybir.AluOpType
AF = mybir.ActivationFunctionType

_cache: dict = {}

# Exposed for test harnesses: the BassKernelResults of the last run.
LAST_RESULT = None


def _build(w0: float, w1: float, w2: float, bias: float) -> bass.Bass:
    nc = bass.Bass()
    # x is host-padded with 3 zero rows at the front of L: padded row r
    # holds true row r-3. This makes every tile a uniform full-128-partition
    # DMA (SWDGE descriptor fan-out degenerates to 1-2 SDMA engines for
    # partition counts != 128) and provides the conv zero-padding for free.
    x = nc.dram_tensor("x", [BPC, L + 3, D], F32, kind="ExternalInput")
    y = nc.dram_tensor("y", [BPC, LP, D], F32, kind="ExternalOutput")

    # Register the conv bias as a const AP so activation(bias=<float>) works
    # (only 0.0/1.0 are pre-registered). Same pattern as Bass.__init__.
    if (F32, bias) not in nc.const_aps.aps:
        bias_t = nc.alloc_sbuf_tensor(f"const-bias", [128, 1], F32)
        nc.gpsimd.memset(bias_t.ap(), bias)
        nc.const_aps.aps[(F32, bias)] = bias_t.ap()
        nc.all_engine_barrier()

    xrow = D              # elements per L-row
    xbat = (L + 3) * D    # elements per (padded) input batch
    ybat = LP * D

    with TileContext(nc) as tc:
        with (
            tc.tile_pool(name="xp", bufs=3) as xp,
            tc.tile_pool(name="cep", bufs=2) as cep,
            tc.tile_pool(name="cop", bufs=2) as cop,
            tc.tile_pool(name="rp", bufs=2) as rp,
            tc.tile_pool(name="op", bufs=2) as op,
        ):
            for b in range(BPC):
                for t in range(T):
                    X = xp.tile([128, (S + 3) * D], F32)
                    # padded source: true rows R0-3..R0+S-1 = padded rows
                    # R0..R0+S+2 where R0 = t*128*S + p*S
                    base = b * xbat + t * 128 * S * xrow
                    nc.gpsimd.dma_start(
                        out=X[:, :],
                        in_=bass.AP(
                            x, base, [[S * xrow, 128], [1, (S + 3) * xrow]]
                        ),
                    )

                    Xv = X[:, :].rearrange("p (r d) -> p r d", d=D)
                    # ce taps: x rows (local) 4j+2, 4j+3, 4j+4   j in [0, JT)
                    xa_e = Xv[:, 2 : 2 + 4 * JT - 3 : 4, :]
                    xb_e = Xv[:, 3 : 3 + 4 * JT - 3 : 4, :]
                    xc_e = Xv[:, 4 : 4 + 4 * JT - 3 : 4, :]
                    # co taps: x rows (local) 4k, 4k+1, 4k+2     k in [0, JT]
                    xa_o = Xv[:, 0 : 4 * JT + 1 : 4, :]
                    xb_o = Xv[:, 1 : 4 * JT + 2 : 4, :]
                    xc_o = Xv[:, 2 : 4 * JT + 3 : 4, :]

                    CE = cep.tile([128, JT * D], F32)
                    CO = cop.tile([128, (JT + 1) * D], F32)
                    ce3 = CE[:, :].rearrange("p (j d) -> p j d", d=D)
                    co3 = CO[:, :].rearrange("p (j d) -> p j d", d=D)

                    # conv (bias folded in): c = w0*xa + w1*xb + w2*xc + bias
                    nc.scalar.activation(ce3, xa_e, AF.Identity, bias=bias, scale=w0)
                    nc.vector.scalar_tensor_tensor(
                        ce3, xb_e, w1, ce3, op0=ALU.mult, op1=ALU.add
                    )
                    nc.vector.scalar_tensor_tensor(
                        ce3, xc_e, w2, ce3, op0=ALU.mult, op1=ALU.add
                    )
                    nc.scalar.activation(co3, xa_o, AF.Identity, bias=bias, scale=w0)
                    nc.vector.scalar_tensor_tensor(
                        co3, xb_o, w1, co3, op0=ALU.mult, op1=ALU.add
                    )
                    nc.vector.scalar_tensor_tensor(
                        co3, xc_o, w2, co3, op0=ALU.mult, op1=ALU.add
                    )
                    if t == 0:
                        # left pool pad: co[-1] = -inf
                        nc.gpsimd.memset(CO[0:1, 0:D], float("-inf"))

                    # maxpool (pre-activation; ELU is monotonic)
                    nc.vector.tensor_tensor(
                        CE[:, :], CE[:, :], CO[:, D:], op=ALU.max
                    )
                    nc.vector.tensor_tensor(
                        CE[:, :], CE[:, :], CO[:, : JT * D], op=ALU.max
                    )

                    # ELU(v) = max(v, exp(min(v,0)) - 1)
                    R = rp.tile([128, JT * D], F32)
                    nc.scalar.activation(R[:, :], CE[:, :], AF.Relu, scale=-1.0)
                    nc.scalar.activation(R[:, :], R[:, :], AF.Exp, scale=-1.0)
                    OUT = op.tile([128, JT * D], F32)
                    nc.vector.scalar_tensor_tensor(
                        OUT[:, :], R[:, :], -1.0, CE[:, :], op0=ALU.add, op1=ALU.max
                    )

                    nc.gpsimd.dma_start(
                        out=bass.AP(
                            y,
                            b * ybat + t * 128 * JT * D,
                            [[JT * D, 128], [1, JT * D]],
                        ),
                        in_=OUT[:, :],
                    )
    return nc


def kernel(x: np.ndarray, w: np.ndarray, b: np.ndarray) -> np.ndarray:
    global LAST_RESULT
    w = np.asarray(w, dtype=np.float32)
    bb = np.asarray(b, dtype=np.float32)
    key = (float(w[0]), float(w[1]), float(w[2]), float(bb[0]))
    if key not in _cache:
        _cache[key] = _build(*key)
    nc = _cache[key]

    x = np.asarray(x, dtype=np.float32)
    xpad = np.zeros((B, L + 3, D), dtype=np.float32)
    xpad[:, 3:, :] = x
    in_maps = [
        {"x": np.ascontiguousarray(xpad[c * BPC : (c + 1) * BPC])}
        for c in range(N_CORES)
    ]
    res = run_bass_kernel_spmd(nc, in_maps, core_ids=list(range(N_CORES)))
    LAST_RESULT = res
    return np.concatenate([r["y"] for r in res.results], axis=0)
